# revision 20
# baseline (speedup 1.0000x reference)
"""Trainium2 Bass kernel for nn_PlasticMultiHeadAttention.

Full computation:
  qkv = hidden @ W_qkv.T + b_qkv            [1, 4096, 3072]
  q,k,v -> [1, 16, 4096, 64]
  attn = softmax(q k^T / 8)
  out  = (attn @ v) * hebbian_diag          [1, 4096, 1024]

Sharding: tensor-parallel over heads, 2 heads per core, 8 cores.
Each core:
  - reads full hidden (transposed, fp16), its slice of W (transposed, fp16,
    with 1/sqrt(8) folded into Wq/Wk and hebbian diag folded into Wv)
  - computes Q^T,K^T [128(=2 heads x 64d), 4096] and V [4096, 2, 65] (65th
    col = ones for softmax row sums)
  - per q-block of 512: S^T tiles via row-packed matmul pairs (contract d=64),
    exp on ScalarE (PSUM->SBUF, fp16), AV matmuls accumulating out^T[65, 512]
    per head (row 64 = softmax denominators)
  - transposes out^T back via PE, normalizes by the denominators (DVE),
    stores [4096, 128] fp32
Host: gathers per-core outputs, concatenates heads.
"""

import numpy as np

S = 4096          # sequence length
E = 1024          # embed dim
H = 16            # heads
D = 64            # head dim
NCORES = 8
HPC = H // NCORES  # heads per core = 2
QB = 512           # q-block size
NQB = S // QB      # 8
KT = S // 128      # 32 k-tiles
EC = E // 128      # 8 e-chunks
TB = S // 512      # 8 token blocks for Q/K projection

# ---- softmax exp split: ScalarE does cols [0, XACT), VectorE the rest ----
# Scores arrive pre-scaled: v = s_raw/(64*BETA*ln2). ScalarE: exp(ACT_SCALE*v).
# VectorE custom ops: t = quartic(v) ~= 2^(BETA*v) (tied a3==a2), then t^8.
BETA = 4.30
ACT_SCALE = float(8 * BETA * np.log(2.0))
HOST_SCALE = float(1.0 / np.sqrt(64 * BETA * np.log(2.0)))
W_FIT = 0.70
XDVE = 112         # columns per head handled by VectorE (0 in q-block 0)

_ww = np.linspace(-W_FIT, W_FIT, 40001)
_tt = 2.0**_ww
_basis = np.stack([_ww / _tt, (_ww**2 + _ww**3 / BETA) / _tt, _ww**4 / _tt], 1)
_wt = np.ones_like(_ww)
for _ in range(150):
    _c, *_r = np.linalg.lstsq(_basis * _wt[:, None], (_tt - 1) / _tt * _wt, rcond=None)
    _res = np.abs((1 + (_basis * _tt[:, None]).dot(_c)) / _tt - 1)
    _wt *= (0.2 + _res / (_res.max() + 1e-30)) ** 0.3
    _wt /= _wt.mean()
PA1 = float(_c[0]) * BETA
PA2 = float(_c[1]) * BETA**2
PA4 = float(_c[2]) * BETA**4

_CACHE = {}


def _register_dve_ops():
    """Idempotently register the exp2 custom DVE ops into concourse.dve_ops."""
    import concourse.dve_ops as dve_ops
    from concourse.dve_spec import Spec, Src0, C0, C1, C2, One, lower, sq
    from concourse.dve_uop import DveOpSpec

    if "EXP2P_ANT" in dve_ops._SUB_OPCODE_FOR_NAME:
        by = {op.name: op for op in dve_ops.OPS}
        return by["EXP2P_ANT"], by["POW8_ANT"]

    v = Src0
    body_a = v * (C2 + v * (C1 + v * (C1 + v * C0))) + One

    def ref_a(in0, in1, s0, s1, imm2):
        vv = in0.astype(np.float32)
        return (
            vv
            * (
                np.float32(imm2)
                + vv * (np.float32(s1) + vv * (np.float32(s1) + vv * np.float32(s0)))
            )
            + np.float32(1.0)
        ).astype(np.float32)

    body_b = sq(sq(sq(Src0)))

    def ref_b(in0, in1, s0, s1, imm2):
        t = in0.astype(np.float32)
        t2 = (t * t).astype(np.float32)
        t4 = (t2 * t2).astype(np.float32)
        return (t4 * t4).astype(np.float32)

    ops = []
    for name, spec in (
        ("EXP2P_ANT", Spec(body=body_a, reference=ref_a)),
        ("POW8_ANT", Spec(body=body_b, reference=ref_b)),
    ):
        row = dve_ops._CUSTOM_DVE_ROW_BASE + len(dve_ops.OPS)
        dve_ops._SUB_OPCODE_FOR_NAME[name] = row
        shas = {}
        for ver in ("v3", "v4"):
            try:
                tmp = DveOpSpec(
                    name=name, opcode=row, uops=lower(spec, ver=ver), rd1_en=False
                )
                shas[ver] = tmp.sha(ver)
            except Exception:
                pass
        op = dve_ops.DveOp(name, spec, subdim=False, uops_sha=shas)
        dve_ops.OPS.append(op)
        dve_ops.CUSTOM_DVE_SPECS[name] = spec
        ops.append(op)
    return ops[0], ops[1]


def _build_program():
    import concourse.bass as bass
    import concourse.mybir as mybir
    import concourse.tile as tile
    from concourse import bacc
    from concourse.bass import ts, ds
    from concourse.masks import make_identity

    f16 = mybir.dt.float16
    f32 = mybir.dt.float32
    AF = mybir.ActivationFunctionType
    ALU = mybir.AluOpType
    op_a, op_b = _register_dve_ops()

    nc = bacc.Bacc(
        "TRN2",
        debug=False,
        enable_asserts=False,
        target_bir_lowering=False,
        num_devices=NCORES,
    )

    hT_d = nc.dram_tensor("hT", [E, S], f16, kind="ExternalInput").ap()
    wqT_d = nc.dram_tensor("wqT", [E, 128], f16, kind="ExternalInput").ap()
    wkT_d = nc.dram_tensor("wkT", [E, 128], f16, kind="ExternalInput").ap()
    wvT_d = nc.dram_tensor("wvT", [E, 2 * 65], f16, kind="ExternalInput").ap()
    bq_d = nc.dram_tensor("bq", [128, 1], f32, kind="ExternalInput").ap()
    bk_d = nc.dram_tensor("bk", [128, 1], f32, kind="ExternalInput").ap()
    out_d = nc.dram_tensor("out", [S, 128], f32, kind="ExternalOutput").ap()

    hT_r = hT_d.rearrange("(eo ei) t -> ei eo t", ei=128)     # [128, 8, 4096]
    wqT_r = wqT_d.rearrange("(eo ei) d -> ei eo d", ei=128)   # [128, 8, 128]
    wkT_r = wkT_d.rearrange("(eo ei) d -> ei eo d", ei=128)
    wvT_r = wvT_d.rearrange("(eo ei) d -> ei eo d", ei=128)   # [128, 8, 130]

    with tile.TileContext(nc) as tc:
        from contextlib import ExitStack

        with ExitStack() as ctx:
            const = ctx.enter_context(tc.tile_pool(name="const", bufs=1))
            qkv_ps = ctx.enter_context(
                tc.tile_pool(name="qkv_ps", bufs=2, space="PSUM")
            )
            st_ps = ctx.enter_context(tc.tile_pool(name="st_ps", bufs=2, space="PSUM"))
            av_ps = ctx.enter_context(tc.tile_pool(name="av_ps", bufs=1, space="PSUM"))
            pt_pool = ctx.enter_context(tc.tile_pool(name="pt", bufs=16))
            ot_pool = ctx.enter_context(tc.tile_pool(name="ot", bufs=4))
            small = ctx.enter_context(tc.tile_pool(name="small", bufs=4))

            # ---------------- persistent SBUF buffers ----------------
            hbuf = const.tile([128, EC, S], f16)        # hidden^T
            wq = const.tile([128, EC, 128], f16)
            wk = const.tile([128, EC, 128], f16)
            wv = const.tile([128, EC, 2 * 65], f16)
            bq_t = const.tile([128, 1], f32)
            bk_t = const.tile([128, 1], f32)
            ident = const.tile([128, 128], f16)
            qt_buf = const.tile([128, S], f16)           # Q^T (2 heads packed)
            kt_buf = const.tile([128, S], f16)           # K^T
            vbuf = const.tile([128, KT, 2, 65], f16)     # V + ones col

            nc.sync.dma_start(wk[:], wkT_r)
            nc.sync.dma_start(wq[:], wqT_r)
            nc.sync.dma_start(bk_t[:], bk_d)
            nc.sync.dma_start(bq_t[:], bq_d)
            for e in range(EC):
                nc.sync.dma_start(hbuf[:, e, ts(0, 512)], hT_r[:, e, ts(0, 512)])
            nc.sync.dma_start(wv[:], wvT_r)
            make_identity(nc, ident[:])
            # ones column for softmax denominators (W_v cols 64/129 are zero,
            # V proj only writes cols 0..63 of each head's 65)
            nc.vector.memset(vbuf[:, :, :, 64:65], 1.0)

            for tb in range(1, TB):
                nc.sync.dma_start(hbuf[:, :, ts(tb, 512)], hT_r[:, :, ts(tb, 512)])

            # ---------------- QKV projection task emitters ----------------
            # Demoted priority: these fill PE idle slots so the S^T -> exp
            # chain (which feeds the bottleneck ScalarE) never starves.
            DEMOTE = 1 << 20

            from contextlib import nullcontext

            def emit_kq(which, tb, demote=True):

                wt, dst, bias = (
                    (wk, kt_buf, bk_t) if which == "K" else (wq, qt_buf, bq_t)
                )
                with nullcontext():
                    ps = qkv_ps.tile([128, 512], f32, tag="qkvps")
                    for e in range(EC):
                        nc.tensor.matmul(
                            ps[:],
                            lhsT=wt[:, e, :],
                            rhs=hbuf[:, e, ts(tb, 512)],
                            start=(e == 0),
                            stop=(e == EC - 1),
                        )
                    nc.vector.tensor_tensor(
                        dst[:, ts(tb, 512)],
                        ps[:],
                        bias[:, 0:1].to_broadcast([128, 512]),
                        ALU.add,
                    )

            def emit_v(tt):
                with nullcontext():
                    ps = qkv_ps.tile([128, 512], f32, tag="qkvps")
                    for e in range(EC):
                        nc.tensor.matmul(
                            ps[:, : 2 * 65],
                            lhsT=hbuf[:, e, ts(tt, 128)],
                            rhs=wv[:, e, :],
                            start=(e == 0),
                            stop=(e == EC - 1),
                        )
                    src = ps[:, : 2 * 65].rearrange("p (h d) -> p h d", h=2)
                    nc.vector.tensor_copy(vbuf[:, tt, :, 0:64], src[:, :, 0:64])

            k_done = [False] * TB
            q_done = [False] * NQB
            v_done = [False] * KT

            def need_k(tb, demote=True):
                if not k_done[tb]:
                    emit_kq("K", tb, demote)
                    k_done[tb] = True

            def need_q(qb, demote=True):
                if not q_done[qb]:
                    emit_kq("Q", qb, demote)
                    q_done[qb] = True

            def need_v(tt):
                if not v_done[tt]:
                    emit_v(tt)
                    v_done[tt] = True

            # ---------------- attention ----------------
            need_k(0, demote=False)
            need_q(0, demote=False)

            for qb in range(NQB):
                need_q(qb)
                if qb + 1 < NQB and qb > 0:
                    need_q(qb + 1)
                av = [
                    av_ps.tile([65, QB], f32, tag=f"av{h}", name=f"av{h}_{qb}")
                    for h in range(2)
                ]
                for k in range(KT):
                    need_k(k // 4)
                    need_v(k)
                    st = st_ps.tile([128, 2, QB], f32, tag="st")
                    # S^T for both heads: row-packed matmul pair (contract d=64)
                    nc.tensor.matmul(
                        st[:, 0, :],
                        lhsT=kt_buf[0:64, ts(k, 128)],
                        rhs=qt_buf[0:64, ts(qb, QB)],
                        start=True,
                        stop=True,
                    )
                    nc.tensor.matmul(
                        st[:, 1, :],
                        lhsT=kt_buf[64:128, ts(k, 128)],
                        rhs=qt_buf[64:128, ts(qb, QB)],
                        start=True,
                        stop=True,
                    )
                    pt = pt_pool.tile([128, 2, QB], f16, tag="pt")
                    xd = 0 if qb == 0 else XDVE  # DVE is busy with V/bias in qb0
                    xa = QB - xd
                    nc.scalar.activation(
                        pt[:, :, 0:xa], st[:, :, 0:xa], AF.Exp, scale=ACT_SCALE
                    )
                    if xd:
                        ta = small.tile([128, 2, XDVE], f32, tag="ta")
                        nc.vector._custom_dve(
                            op_a,
                            out=ta[:],
                            in0=st[:, :, xa:QB],
                            s0=PA4,
                            s1=PA2,
                            imm2=PA1,
                        )
                        nc.vector._custom_dve(op_b, out=pt[:, :, xa:QB], in0=ta[:])
                    for h in range(2):
                        nc.tensor.matmul(
                            av[h][:],
                            lhsT=vbuf[:, k, h, :],
                            rhs=pt[:, h, :],
                            start=(k == 0),
                            stop=(k == KT - 1),
                            skip_group_check=True,
                        )
                # epilogue: transpose + normalize + store
                for h in range(2):
                    ot = ot_pool.tile([128, QB], f16, tag="ot")
                    nc.vector.tensor_copy(ot[0:65, :], av[h][:])
                    for i in range(QB // 128):
                        tp = small.tile([128, 128], f16, tag="tp")
                        nc.sync.dma_start_transpose(tp[:], ot[:, ts(i, 128)])
                        rc = small.tile([128, 1], f32, tag="rc")
                        nc.vector.reciprocal(rc[:], tp[:, 64:65])
                        res = small.tile([128, 64], f32, tag="res")
                        nc.vector.tensor_tensor(
                            res[:], tp[:, 0:64], rc[:, 0:1].to_broadcast([128, 64]), ALU.mult
                        )
                        nc.sync.dma_start(
                            out_d[ds(qb * QB + i * 128, 128), ds(h * 64, 64)],
                            res[:],
                        )

    nc.compile()
    return nc


def _host_prep(hidden_states, W_qkv, b_qkv, hebbian_trace):
    """Build per-core input maps (all fp16 except biases)."""
    h = np.asarray(hidden_states, np.float32).reshape(S, E)
    W = np.asarray(W_qkv, np.float32)
    b = np.asarray(b_qkv, np.float32)
    heb = np.asarray(hebbian_trace, np.float32)

    sc = HOST_SCALE  # scores become v = s_raw/(64*BETA*ln2); exp = exp(ACT_SCALE*v)
    hT = np.ascontiguousarray(h.T).astype(np.float16)  # [E, S]

    Wq, Wk, Wv = W[0:E], W[E : 2 * E], W[2 * E :]
    bq, bk, bv = b[0:E], b[E : 2 * E], b[2 * E :]
    hebd = np.ascontiguousarray(
        heb[:, np.arange(D), np.arange(D)]
    )  # [H, D] diagonal

    in_maps = []
    bv_scaled_full = (bv.reshape(H, D) * hebd).reshape(E)  # added on host at end
    for c in range(NCORES):
        hs = [2 * c, 2 * c + 1]
        rows = np.concatenate([np.arange(hh * D, (hh + 1) * D) for hh in hs])
        wq_c = (Wq[rows] * sc).astype(np.float32)           # [128, E]
        wk_c = (Wk[rows] * sc).astype(np.float32)
        wv_c = Wv[rows] * hebd[hs].reshape(128)[:, None]    # [128, E] hebbian folded
        # V layout per head: 64 dims + 1 zero col (becomes ones via memset)
        wv_ext = np.zeros((2 * 65, E), np.float32)
        wv_ext[0:64] = wv_c[0:64]
        wv_ext[65:129] = wv_c[64:128]
        in_maps.append(
            {
                "hT": hT,
                "wqT": np.ascontiguousarray(wq_c.T).astype(np.float16),
                "wkT": np.ascontiguousarray(wk_c.T).astype(np.float16),
                "wvT": np.ascontiguousarray(wv_ext.T).astype(np.float16),
                "bq": (bq[rows] * sc).astype(np.float32).reshape(128, 1),
                "bk": (bk[rows] * sc).astype(np.float32).reshape(128, 1),
            }
        )
    return in_maps, bv_scaled_full


def _get_nc():
    if "nc" not in _CACHE:
        _CACHE["nc"] = _build_program()
    return _CACHE["nc"]


def kernel(hidden_states, W_qkv, b_qkv, hebbian_trace, _trace=False, _tmpdir=None):
    from concourse.bass_utils import run_bass_kernel_spmd

    in_maps, bv_scaled = _host_prep(hidden_states, W_qkv, b_qkv, hebbian_trace)
    nc = _get_nc()
    res = run_bass_kernel_spmd(
        nc,
        in_maps,
        core_ids=list(range(NCORES)),
        trace=_trace,
        tmpdir=_tmpdir,
    )
    outs = [res.results[c]["out"] for c in range(NCORES)]
    full = np.concatenate(outs, axis=1).astype(np.float32)  # [S, E]
    full = full + bv_scaled[None, :]
    out = full.reshape(1, S, E)
    if _trace:
        return out, res
    return out


# revision 21
# speedup vs baseline: 1.2182x; 1.2182x over previous
"""Trainium2 Bass kernel for nn_PlasticMultiHeadAttention.

Full computation:
  qkv = hidden @ W_qkv.T + b_qkv            [1, 4096, 3072]
  q,k,v -> [1, 16, 4096, 64]
  attn = softmax(q k^T / 8)
  out  = (attn @ v) * hebbian_diag          [1, 4096, 1024]

Sharding: tensor-parallel over heads, 2 heads per core, 8 cores.
Each core:
  - reads full hidden (transposed, fp16), its slice of W (transposed, fp16,
    with 1/sqrt(8) folded into Wq/Wk and hebbian diag folded into Wv)
  - computes Q^T,K^T [128(=2 heads x 64d), 4096] and V [4096, 2, 65] (65th
    col = ones for softmax row sums)
  - per q-block of 512: S^T tiles via row-packed matmul pairs (contract d=64),
    exp on ScalarE (PSUM->SBUF, fp16), AV matmuls accumulating out^T[65, 512]
    per head (row 64 = softmax denominators)
  - transposes out^T back via PE, normalizes by the denominators (DVE),
    stores [4096, 128] fp32
Host: gathers per-core outputs, concatenates heads.
"""

import numpy as np

S = 4096          # sequence length
E = 1024          # embed dim
H = 16            # heads
D = 64            # head dim
NCORES = 8
HPC = H // NCORES  # heads per core = 2
QB = 512           # q-block size
NQB = S // QB      # 8
KT = S // 128      # 32 k-tiles
EC = E // 128      # 8 e-chunks
TB = S // 512      # 8 token blocks for Q/K projection

# ---- softmax exp split: ScalarE does cols [0, XACT), VectorE the rest ----
# Scores arrive pre-scaled: v = s_raw/(64*BETA*ln2). ScalarE: exp(ACT_SCALE*v).
# VectorE custom ops: t = quartic(v) ~= 2^(BETA*v) (tied a3==a2), then t^8.
BETA = 4.30
ACT_SCALE = float(8 * BETA * np.log(2.0))
HOST_SCALE = float(1.0 / np.sqrt(64 * BETA * np.log(2.0)))
W_FIT = 0.70
XDVE = 112         # columns per head handled by VectorE (0 in q-block 0)

_ww = np.linspace(-W_FIT, W_FIT, 40001)
_tt = 2.0**_ww
_basis = np.stack([_ww / _tt, (_ww**2 + _ww**3 / BETA) / _tt, _ww**4 / _tt], 1)
_wt = np.ones_like(_ww)
for _ in range(150):
    _c, *_r = np.linalg.lstsq(_basis * _wt[:, None], (_tt - 1) / _tt * _wt, rcond=None)
    _res = np.abs((1 + (_basis * _tt[:, None]).dot(_c)) / _tt - 1)
    _wt *= (0.2 + _res / (_res.max() + 1e-30)) ** 0.3
    _wt /= _wt.mean()
PA1 = float(_c[0]) * BETA
PA2 = float(_c[1]) * BETA**2
PA4 = float(_c[2]) * BETA**4

_CACHE = {}


def _register_dve_ops():
    """Idempotently register the exp2 custom DVE ops into concourse.dve_ops."""
    import concourse.dve_ops as dve_ops
    from concourse.dve_spec import Spec, Src0, C0, C1, C2, One, lower, sq
    from concourse.dve_uop import DveOpSpec

    if "EXP2P_ANT" in dve_ops._SUB_OPCODE_FOR_NAME:
        by = {op.name: op for op in dve_ops.OPS}
        return by["EXP2P_ANT"], by["POW8_ANT"]

    v = Src0
    body_a = v * (C2 + v * (C1 + v * (C1 + v * C0))) + One

    def ref_a(in0, in1, s0, s1, imm2):
        vv = in0.astype(np.float32)
        return (
            vv
            * (
                np.float32(imm2)
                + vv * (np.float32(s1) + vv * (np.float32(s1) + vv * np.float32(s0)))
            )
            + np.float32(1.0)
        ).astype(np.float32)

    body_b = sq(sq(sq(Src0)))

    def ref_b(in0, in1, s0, s1, imm2):
        t = in0.astype(np.float32)
        t2 = (t * t).astype(np.float32)
        t4 = (t2 * t2).astype(np.float32)
        return (t4 * t4).astype(np.float32)

    ops = []
    for name, spec in (
        ("EXP2P_ANT", Spec(body=body_a, reference=ref_a)),
        ("POW8_ANT", Spec(body=body_b, reference=ref_b)),
    ):
        row = dve_ops._CUSTOM_DVE_ROW_BASE + len(dve_ops.OPS)
        dve_ops._SUB_OPCODE_FOR_NAME[name] = row
        shas = {}
        for ver in ("v3", "v4"):
            try:
                tmp = DveOpSpec(
                    name=name, opcode=row, uops=lower(spec, ver=ver), rd1_en=False
                )
                shas[ver] = tmp.sha(ver)
            except Exception:
                pass
        op = dve_ops.DveOp(name, spec, subdim=False, uops_sha=shas)
        dve_ops.OPS.append(op)
        dve_ops.CUSTOM_DVE_SPECS[name] = spec
        ops.append(op)
    return ops[0], ops[1]


def _build_program():
    import concourse.bass as bass
    import concourse.mybir as mybir
    import concourse.tile as tile
    from concourse import bacc
    from concourse.bass import ts, ds
    from concourse.masks import make_identity

    f16 = mybir.dt.float16
    f32 = mybir.dt.float32
    AF = mybir.ActivationFunctionType
    ALU = mybir.AluOpType
    op_a, op_b = _register_dve_ops()

    nc = bacc.Bacc(
        "TRN2",
        debug=False,
        enable_asserts=False,
        target_bir_lowering=False,
        num_devices=NCORES,
    )

    hT_d = nc.dram_tensor("hT", [E, S], f16, kind="ExternalInput").ap()
    wqT_d = nc.dram_tensor("wqT", [E, 128], f16, kind="ExternalInput").ap()
    wkT_d = nc.dram_tensor("wkT", [E, 128], f16, kind="ExternalInput").ap()
    wvT_d = nc.dram_tensor("wvT", [E, 2 * 65], f16, kind="ExternalInput").ap()
    bq_d = nc.dram_tensor("bq", [128, 1], f32, kind="ExternalInput").ap()
    bk_d = nc.dram_tensor("bk", [128, 1], f32, kind="ExternalInput").ap()
    out_d = nc.dram_tensor("out", [S, 128], f32, kind="ExternalOutput").ap()

    hT_r = hT_d.rearrange("(eo ei) t -> ei eo t", ei=128)     # [128, 8, 4096]
    wqT_r = wqT_d.rearrange("(eo ei) d -> ei eo d", ei=128)   # [128, 8, 128]
    wkT_r = wkT_d.rearrange("(eo ei) d -> ei eo d", ei=128)
    wvT_r = wvT_d.rearrange("(eo ei) d -> ei eo d", ei=128)   # [128, 8, 130]

    with tile.TileContext(nc) as tc:
        from contextlib import ExitStack

        with ExitStack() as ctx:
            const = ctx.enter_context(tc.tile_pool(name="const", bufs=1))
            qkv_ps = ctx.enter_context(
                tc.tile_pool(name="qkv_ps", bufs=1, space="PSUM")
            )
            st_ps = ctx.enter_context(tc.tile_pool(name="st_ps", bufs=2, space="PSUM"))
            av_ps = ctx.enter_context(tc.tile_pool(name="av_ps", bufs=1, space="PSUM"))
            tr_ps = ctx.enter_context(tc.tile_pool(name="tr_ps", bufs=1, space="PSUM"))
            pt_pool = ctx.enter_context(tc.tile_pool(name="pt", bufs=16))
            ot_pool = ctx.enter_context(tc.tile_pool(name="ot", bufs=4))
            small = ctx.enter_context(tc.tile_pool(name="small", bufs=4))

            # ---------------- persistent SBUF buffers ----------------
            hbuf = const.tile([128, EC, S], f16)        # hidden^T
            wq = const.tile([128, EC, 128], f16)
            wk = const.tile([128, EC, 128], f16)
            wv = const.tile([128, EC, 2 * 65], f16)
            bq_t = const.tile([128, 1], f32)
            bk_t = const.tile([128, 1], f32)
            ident = const.tile([128, 128], f16)
            qt_buf = const.tile([128, S], f16)           # Q^T (2 heads packed)
            kt_buf = const.tile([128, S], f16)           # K^T
            vbuf = const.tile([128, KT, 2, 65], f16)     # V + ones col

            nc.sync.dma_start(wk[:], wkT_r)
            nc.sync.dma_start(wq[:], wqT_r)
            nc.sync.dma_start(bk_t[:], bk_d)
            nc.sync.dma_start(bq_t[:], bq_d)
            nc.sync.dma_start(hbuf[:, :, ts(0, 512)], hT_r[:, :, ts(0, 512)])
            nc.sync.dma_start(wv[:], wvT_r)
            make_identity(nc, ident[:])
            # ones column for softmax denominators (W_v cols 64/129 are zero,
            # V proj only writes cols 0..63 of each head's 65)
            nc.vector.memset(vbuf[:, :, :, 64:65], 1.0)

            for tb in range(1, TB):
                nc.sync.dma_start(hbuf[:, :, ts(tb, 512)], hT_r[:, :, ts(tb, 512)])

            # ---------------- QKV projection task emitters ----------------
            # Demoted priority: these fill PE idle slots so the S^T -> exp
            # chain (which feeds the bottleneck ScalarE) never starves.
            DEMOTE = 1 << 20

            from contextlib import nullcontext

            def emit_kq(which, tb, demote=True):

                wt, dst, bias = (
                    (wk, kt_buf, bk_t) if which == "K" else (wq, qt_buf, bq_t)
                )
                with nullcontext():
                    ps = qkv_ps.tile([128, 512], f32, tag="qkvps")
                    for e in range(EC):
                        nc.tensor.matmul(
                            ps[:],
                            lhsT=wt[:, e, :],
                            rhs=hbuf[:, e, ts(tb, 512)],
                            start=(e == 0),
                            stop=(e == EC - 1),
                        )
                    nc.vector.tensor_tensor(
                        dst[:, ts(tb, 512)],
                        ps[:],
                        bias[:, 0:1].to_broadcast([128, 512]),
                        ALU.add,
                    )

            def emit_v(tt):
                with nullcontext():
                    ps = qkv_ps.tile([128, 512], f32, tag="qkvps")
                    for e in range(EC):
                        nc.tensor.matmul(
                            ps[:, : 2 * 65],
                            lhsT=hbuf[:, e, ts(tt, 128)],
                            rhs=wv[:, e, :],
                            start=(e == 0),
                            stop=(e == EC - 1),
                        )
                    src = ps[:, : 2 * 65].rearrange("p (h d) -> p h d", h=2)
                    nc.vector.tensor_copy(vbuf[:, tt, :, 0:64], src[:, :, 0:64])

            k_done = [False] * TB
            q_done = [False] * NQB
            v_done = [False] * KT

            def need_k(tb, demote=True):
                if not k_done[tb]:
                    emit_kq("K", tb, demote)
                    k_done[tb] = True

            def need_q(qb, demote=True):
                if not q_done[qb]:
                    emit_kq("Q", qb, demote)
                    q_done[qb] = True

            def need_v(tt):
                if not v_done[tt]:
                    emit_v(tt)
                    v_done[tt] = True

            # ---------------- attention ----------------
            need_k(0, demote=False)
            need_q(0, demote=False)

            for qb in range(NQB):
                need_q(qb)
                if qb + 1 < NQB and qb > 0:
                    need_q(qb + 1)
                av = [
                    av_ps.tile([65, QB], f32, tag=f"av{h}", name=f"av{h}_{qb}")
                    for h in range(2)
                ]
                for k in range(KT):
                    need_k(k // 4)
                    need_v(k)
                    st = st_ps.tile([128, 2, QB], f32, tag="st")
                    # S^T for both heads: row-packed matmul pair (contract d=64)
                    nc.tensor.matmul(
                        st[:, 0, :],
                        lhsT=kt_buf[0:64, ts(k, 128)],
                        rhs=qt_buf[0:64, ts(qb, QB)],
                        start=True,
                        stop=True,
                    )
                    nc.tensor.matmul(
                        st[:, 1, :],
                        lhsT=kt_buf[64:128, ts(k, 128)],
                        rhs=qt_buf[64:128, ts(qb, QB)],
                        start=True,
                        stop=True,
                    )
                    pt = pt_pool.tile([128, 2, QB], f16, tag="pt")
                    xd = 0 if qb == 0 else XDVE  # DVE is busy with V/bias in qb0
                    xa = QB - xd
                    nc.scalar.activation(
                        pt[:, :, 0:xa], st[:, :, 0:xa], AF.Exp, scale=ACT_SCALE
                    )
                    if xd:
                        ta = small.tile([128, 2, XDVE], f32, tag="ta")
                        nc.vector._custom_dve(
                            op_a,
                            out=ta[:],
                            in0=st[:, :, xa:QB],
                            s0=PA4,
                            s1=PA2,
                            imm2=PA1,
                        )
                        nc.vector._custom_dve(op_b, out=pt[:, :, xa:QB], in0=ta[:])
                    for h in range(2):
                        nc.tensor.matmul(
                            av[h][:],
                            lhsT=vbuf[:, k, h, :],
                            rhs=pt[:, h, :],
                            start=(k == 0),
                            stop=(k == KT - 1),
                            skip_group_check=True,
                        )
                # epilogue: transpose + normalize + store
                for h in range(2):
                    ot = ot_pool.tile([65, QB], f16, tag="ot")
                    nc.vector.tensor_copy(ot[:], av[h][:])
                    for i in range(QB // 128):
                        tp = tr_ps.tile([128, 65], f16, tag="tp")
                        nc.tensor.transpose(tp[:], ot[:, ts(i, 128)], ident[:65, :65])
                        rc = small.tile([128, 1], f32, tag="rc")
                        nc.vector.reciprocal(rc[:], tp[:, 64:65])
                        res = small.tile([128, 64], f32, tag="res")
                        nc.vector.tensor_tensor(
                            res[:], tp[:, 0:64], rc[:, 0:1].to_broadcast([128, 64]), ALU.mult
                        )
                        nc.sync.dma_start(
                            out_d[ds(qb * QB + i * 128, 128), ds(h * 64, 64)],
                            res[:],
                        )

    nc.compile()
    return nc


def _host_prep(hidden_states, W_qkv, b_qkv, hebbian_trace):
    """Build per-core input maps (all fp16 except biases)."""
    h = np.asarray(hidden_states, np.float32).reshape(S, E)
    W = np.asarray(W_qkv, np.float32)
    b = np.asarray(b_qkv, np.float32)
    heb = np.asarray(hebbian_trace, np.float32)

    sc = HOST_SCALE  # scores become v = s_raw/(64*BETA*ln2); exp = exp(ACT_SCALE*v)
    hT = np.ascontiguousarray(h.T).astype(np.float16)  # [E, S]

    Wq, Wk, Wv = W[0:E], W[E : 2 * E], W[2 * E :]
    bq, bk, bv = b[0:E], b[E : 2 * E], b[2 * E :]
    hebd = np.ascontiguousarray(
        heb[:, np.arange(D), np.arange(D)]
    )  # [H, D] diagonal

    in_maps = []
    bv_scaled_full = (bv.reshape(H, D) * hebd).reshape(E)  # added on host at end
    for c in range(NCORES):
        hs = [2 * c, 2 * c + 1]
        rows = np.concatenate([np.arange(hh * D, (hh + 1) * D) for hh in hs])
        wq_c = (Wq[rows] * sc).astype(np.float32)           # [128, E]
        wk_c = (Wk[rows] * sc).astype(np.float32)
        wv_c = Wv[rows] * hebd[hs].reshape(128)[:, None]    # [128, E] hebbian folded
        # V layout per head: 64 dims + 1 zero col (becomes ones via memset)
        wv_ext = np.zeros((2 * 65, E), np.float32)
        wv_ext[0:64] = wv_c[0:64]
        wv_ext[65:129] = wv_c[64:128]
        in_maps.append(
            {
                "hT": hT,
                "wqT": np.ascontiguousarray(wq_c.T).astype(np.float16),
                "wkT": np.ascontiguousarray(wk_c.T).astype(np.float16),
                "wvT": np.ascontiguousarray(wv_ext.T).astype(np.float16),
                "bq": (bq[rows] * sc).astype(np.float32).reshape(128, 1),
                "bk": (bk[rows] * sc).astype(np.float32).reshape(128, 1),
            }
        )
    return in_maps, bv_scaled_full


def _get_nc():
    if "nc" not in _CACHE:
        _CACHE["nc"] = _build_program()
    return _CACHE["nc"]


def kernel(hidden_states, W_qkv, b_qkv, hebbian_trace, _trace=False, _tmpdir=None):
    from concourse.bass_utils import run_bass_kernel_spmd

    in_maps, bv_scaled = _host_prep(hidden_states, W_qkv, b_qkv, hebbian_trace)
    nc = _get_nc()
    res = run_bass_kernel_spmd(
        nc,
        in_maps,
        core_ids=list(range(NCORES)),
        trace=_trace,
        tmpdir=_tmpdir,
    )
    outs = [res.results[c]["out"] for c in range(NCORES)]
    full = np.concatenate(outs, axis=1).astype(np.float32)  # [S, E]
    full = full + bv_scaled[None, :]
    out = full.reshape(1, S, E)
    if _trace:
        return out, res
    return out


# revision 23
# speedup vs baseline: 1.2246x; 1.0052x over previous
"""Trainium2 Bass kernel for nn_PlasticMultiHeadAttention.

Full computation:
  qkv = hidden @ W_qkv.T + b_qkv            [1, 4096, 3072]
  q,k,v -> [1, 16, 4096, 64]
  attn = softmax(q k^T / 8)
  out  = (attn @ v) * hebbian_diag          [1, 4096, 1024]

Sharding: tensor-parallel over heads, 2 heads per core, 8 cores.
Each core:
  - reads full hidden (transposed, fp16), its slice of W (transposed, fp16,
    with 1/sqrt(8) folded into Wq/Wk and hebbian diag folded into Wv)
  - computes Q^T,K^T [128(=2 heads x 64d), 4096] and V [4096, 2, 65] (65th
    col = ones for softmax row sums)
  - per q-block of 512: S^T tiles via row-packed matmul pairs (contract d=64),
    exp on ScalarE (PSUM->SBUF, fp16), AV matmuls accumulating out^T[65, 512]
    per head (row 64 = softmax denominators)
  - transposes out^T back via PE, normalizes by the denominators (DVE),
    stores [4096, 128] fp32
Host: gathers per-core outputs, concatenates heads.
"""

import numpy as np

S = 4096          # sequence length
E = 1024          # embed dim
H = 16            # heads
D = 64            # head dim
NCORES = 8
HPC = H // NCORES  # heads per core = 2
QB = 512           # q-block size
NQB = S // QB      # 8
KT = S // 128      # 32 k-tiles
EC = E // 128      # 8 e-chunks
TB = S // 512      # 8 token blocks for Q/K projection

# ---- softmax exp split: ScalarE does cols [0, XACT), VectorE the rest ----
# Scores arrive pre-scaled: v = s_raw/(64*BETA*ln2). ScalarE: exp(ACT_SCALE*v).
# VectorE custom ops: t = quartic(v) ~= 2^(BETA*v) (tied a3==a2), then t^8.
BETA = 4.30
ACT_SCALE = float(8 * BETA * np.log(2.0))
HOST_SCALE = float(1.0 / np.sqrt(64 * BETA * np.log(2.0)))
W_FIT = 0.70
XDVE = 112         # columns per head handled by VectorE (0 in q-block 0)

_ww = np.linspace(-W_FIT, W_FIT, 40001)
_tt = 2.0**_ww
_basis = np.stack([_ww / _tt, (_ww**2 + _ww**3 / BETA) / _tt, _ww**4 / _tt], 1)
_wt = np.ones_like(_ww)
for _ in range(150):
    _c, *_r = np.linalg.lstsq(_basis * _wt[:, None], (_tt - 1) / _tt * _wt, rcond=None)
    _res = np.abs((1 + (_basis * _tt[:, None]).dot(_c)) / _tt - 1)
    _wt *= (0.2 + _res / (_res.max() + 1e-30)) ** 0.3
    _wt /= _wt.mean()
PA1 = float(_c[0]) * BETA
PA2 = float(_c[1]) * BETA**2
PA4 = float(_c[2]) * BETA**4

_CACHE = {}


def _register_dve_ops():
    """Idempotently register the exp2 custom DVE ops into concourse.dve_ops."""
    import concourse.dve_ops as dve_ops
    from concourse.dve_spec import Spec, Src0, C0, C1, C2, One, lower, sq
    from concourse.dve_uop import DveOpSpec

    if "EXP2P_ANT" in dve_ops._SUB_OPCODE_FOR_NAME:
        by = {op.name: op for op in dve_ops.OPS}
        return by["EXP2P_ANT"], by["POW8_ANT"]

    v = Src0
    body_a = v * (C2 + v * (C1 + v * (C1 + v * C0))) + One

    def ref_a(in0, in1, s0, s1, imm2):
        vv = in0.astype(np.float32)
        return (
            vv
            * (
                np.float32(imm2)
                + vv * (np.float32(s1) + vv * (np.float32(s1) + vv * np.float32(s0)))
            )
            + np.float32(1.0)
        ).astype(np.float32)

    body_b = sq(sq(sq(Src0)))

    def ref_b(in0, in1, s0, s1, imm2):
        t = in0.astype(np.float32)
        t2 = (t * t).astype(np.float32)
        t4 = (t2 * t2).astype(np.float32)
        return (t4 * t4).astype(np.float32)

    ops = []
    for name, spec in (
        ("EXP2P_ANT", Spec(body=body_a, reference=ref_a)),
        ("POW8_ANT", Spec(body=body_b, reference=ref_b)),
    ):
        row = dve_ops._CUSTOM_DVE_ROW_BASE + len(dve_ops.OPS)
        dve_ops._SUB_OPCODE_FOR_NAME[name] = row
        shas = {}
        for ver in ("v3", "v4"):
            try:
                tmp = DveOpSpec(
                    name=name, opcode=row, uops=lower(spec, ver=ver), rd1_en=False
                )
                shas[ver] = tmp.sha(ver)
            except Exception:
                pass
        op = dve_ops.DveOp(name, spec, subdim=False, uops_sha=shas)
        dve_ops.OPS.append(op)
        dve_ops.CUSTOM_DVE_SPECS[name] = spec
        ops.append(op)
    return ops[0], ops[1]


def _build_program():
    import concourse.bass as bass
    import concourse.mybir as mybir
    import concourse.tile as tile
    from concourse import bacc
    from concourse.bass import ts, ds
    from concourse.masks import make_identity

    f16 = mybir.dt.float16
    f32 = mybir.dt.float32
    AF = mybir.ActivationFunctionType
    ALU = mybir.AluOpType
    op_a, op_b = _register_dve_ops()

    nc = bacc.Bacc(
        "TRN2",
        debug=False,
        enable_asserts=False,
        target_bir_lowering=False,
        num_devices=NCORES,
    )

    hT_d = nc.dram_tensor("hT", [E, S], f16, kind="ExternalInput").ap()
    wqT_d = nc.dram_tensor("wqT", [E, 128], f16, kind="ExternalInput").ap()
    wkT_d = nc.dram_tensor("wkT", [E, 128], f16, kind="ExternalInput").ap()
    wvT_d = nc.dram_tensor("wvT", [E, 2 * 65], f16, kind="ExternalInput").ap()
    bq_d = nc.dram_tensor("bq", [128, 1], f32, kind="ExternalInput").ap()
    bk_d = nc.dram_tensor("bk", [128, 1], f32, kind="ExternalInput").ap()
    out_d = nc.dram_tensor("out", [S, 128], f32, kind="ExternalOutput").ap()

    hT_r = hT_d.rearrange("(eo ei) t -> ei eo t", ei=128)     # [128, 8, 4096]
    wqT_r = wqT_d.rearrange("(eo ei) d -> ei eo d", ei=128)   # [128, 8, 128]
    wkT_r = wkT_d.rearrange("(eo ei) d -> ei eo d", ei=128)
    wvT_r = wvT_d.rearrange("(eo ei) d -> ei eo d", ei=128)   # [128, 8, 130]

    with tile.TileContext(nc) as tc:
        from contextlib import ExitStack

        with ExitStack() as ctx:
            const = ctx.enter_context(tc.tile_pool(name="const", bufs=1))
            qkv_ps = ctx.enter_context(
                tc.tile_pool(name="qkv_ps", bufs=1, space="PSUM")
            )
            st_ps = ctx.enter_context(tc.tile_pool(name="st_ps", bufs=2, space="PSUM"))
            av_ps = ctx.enter_context(tc.tile_pool(name="av_ps", bufs=1, space="PSUM"))
            tr_ps = ctx.enter_context(tc.tile_pool(name="tr_ps", bufs=1, space="PSUM"))
            pt_pool = ctx.enter_context(tc.tile_pool(name="pt", bufs=16))
            ot_pool = ctx.enter_context(tc.tile_pool(name="ot", bufs=4))
            small = ctx.enter_context(tc.tile_pool(name="small", bufs=4))

            # ---------------- persistent SBUF buffers ----------------
            hbuf = const.tile([128, EC, S], f16)        # hidden^T
            wq = const.tile([128, EC, 128], f16)
            wk = const.tile([128, EC, 128], f16)
            wv = const.tile([128, EC, 2 * 65], f16)
            bq_t = const.tile([128, 1], f32)
            bk_t = const.tile([128, 1], f32)
            ident = const.tile([128, 128], f16)
            qt_buf = const.tile([128, S], f16)           # Q^T (2 heads packed)
            kt_buf = const.tile([128, S], f16)           # K^T
            vbuf = const.tile([128, KT, 2, 65], f16)     # V + ones col

            nc.sync.dma_start(wk[:], wkT_r)
            nc.sync.dma_start(wq[:], wqT_r)
            nc.sync.dma_start(bk_t[:], bk_d)
            nc.sync.dma_start(bq_t[:], bq_d)
            for e in range(EC):
                nc.sync.dma_start(hbuf[:, e, ts(0, 512)], hT_r[:, e, ts(0, 512)])
            nc.sync.dma_start(wv[:], wvT_r)
            make_identity(nc, ident[:])
            # ones column for softmax denominators (W_v cols 64/129 are zero,
            # V proj only writes cols 0..63 of each head's 65)
            nc.vector.memset(vbuf[:, :, :, 64:65], 1.0)

            for tb in range(1, TB):
                nc.sync.dma_start(hbuf[:, :, ts(tb, 512)], hT_r[:, :, ts(tb, 512)])

            # ---------------- QKV projection task emitters ----------------
            # Demoted priority: these fill PE idle slots so the S^T -> exp
            # chain (which feeds the bottleneck ScalarE) never starves.
            DEMOTE = 1 << 20

            from contextlib import nullcontext

            kq_state = {}  # (which, tb) -> [ps_tile, next_e]

            def kq_step(which, tb, n_mm):
                # emit up to n_mm matmuls of the (which, tb) projection group;
                # finishes with the bias-add copy when all EC chunks are done.
                wt, dst, bias = (
                    (wk, kt_buf, bk_t) if which == "K" else (wq, qt_buf, bq_t)
                )
                if (which, tb) not in kq_state:
                    ps_new = qkv_ps.tile(
                        [128, 512], f32, tag="qkvps", name=f"kq_{which}_{tb}"
                    )
                    kq_state[(which, tb)] = [ps_new, 0]
                st_ = kq_state[(which, tb)]
                ps, e0 = st_
                for e in range(e0, min(e0 + n_mm, EC)):
                    nc.tensor.matmul(
                        ps[:],
                        lhsT=wt[:, e, :],
                        rhs=hbuf[:, e, ts(tb, 512)],
                        start=(e == 0),
                        stop=(e == EC - 1),
                    )
                st_[1] = min(e0 + n_mm, EC)
                if st_[1] == EC:
                    nc.vector.tensor_tensor(
                        dst[:, ts(tb, 512)],
                        ps[:],
                        bias[:, 0:1].to_broadcast([128, 512]),
                        ALU.add,
                    )
                    st_[1] = EC + 1  # done

            def kq_done(which, tb):
                return kq_state.get((which, tb), [None, 0])[1] > EC

            def emit_kq(which, tb, demote=True):
                kq_step(which, tb, EC)

            def emit_v(tt):
                with nullcontext():
                    ps = qkv_ps.tile([128, 512], f32, tag="qkvps")
                    for e in range(EC):
                        nc.tensor.matmul(
                            ps[:, : 2 * 65],
                            lhsT=hbuf[:, e, ts(tt, 128)],
                            rhs=wv[:, e, :],
                            start=(e == 0),
                            stop=(e == EC - 1),
                        )
                    src = ps[:, : 2 * 65].rearrange("p (h d) -> p h d", h=2)
                    nc.vector.tensor_copy(vbuf[:, tt, :, 0:64], src[:, :, 0:64])

            k_done = [False] * TB
            q_done = [False] * NQB
            v_done = [False] * KT

            def need_k(tb, demote=True):
                if not kq_done("K", tb):
                    kq_step("K", tb, EC)
                k_done[tb] = True

            def need_q(qb, demote=True):
                if not kq_done("Q", qb):
                    kq_step("Q", qb, EC)
                q_done[qb] = True

            def need_v(tt):
                if not v_done[tt]:
                    emit_v(tt)
                    v_done[tt] = True

            # ---------------- attention ----------------
            need_k(0, demote=False)
            need_q(0, demote=False)

            for qb in range(NQB):
                need_q(qb)
                if qb + 1 < NQB and qb > 0:
                    need_q(qb + 1)
                av = [
                    av_ps.tile([65, QB], f32, tag=f"av{h}", name=f"av{h}_{qb}")
                    for h in range(2)
                ]
                for k in range(KT):
                    need_k(k // 4)
                    need_v(k)
                    # pump pending projection groups 2 matmuls at a time so PE
                    # detours stay short and the exp pipeline is never starved
                    if qb == 0 and k + 4 < KT and not kq_done("K", (k + 4) // 4):
                        kq_step("K", (k + 4) // 4, 2)
                        k_done[(k + 4) // 4] = True
                    if qb + 1 < NQB and not kq_done("Q", qb + 1) and (qb > 0 or k >= 24):
                        kq_step("Q", qb + 1, 2)
                        q_done[qb + 1] = True
                    st = st_ps.tile([128, 2, QB], f32, tag="st")
                    # S^T for both heads: row-packed matmul pair (contract d=64)
                    nc.tensor.matmul(
                        st[:, 0, :],
                        lhsT=kt_buf[0:64, ts(k, 128)],
                        rhs=qt_buf[0:64, ts(qb, QB)],
                        start=True,
                        stop=True,
                    )
                    nc.tensor.matmul(
                        st[:, 1, :],
                        lhsT=kt_buf[64:128, ts(k, 128)],
                        rhs=qt_buf[64:128, ts(qb, QB)],
                        start=True,
                        stop=True,
                    )
                    pt = pt_pool.tile([128, 2, QB], f16, tag="pt")
                    xd = 0 if qb == 0 else XDVE  # DVE is busy with V/bias in qb0
                    xa = QB - xd
                    nc.scalar.activation(
                        pt[:, :, 0:xa], st[:, :, 0:xa], AF.Exp, scale=ACT_SCALE
                    )
                    if xd:
                        ta = small.tile([128, 2, XDVE], f32, tag="ta")
                        nc.vector._custom_dve(
                            op_a,
                            out=ta[:],
                            in0=st[:, :, xa:QB],
                            s0=PA4,
                            s1=PA2,
                            imm2=PA1,
                        )
                        nc.vector._custom_dve(op_b, out=pt[:, :, xa:QB], in0=ta[:])
                    for h in range(2):
                        nc.tensor.matmul(
                            av[h][:],
                            lhsT=vbuf[:, k, h, :],
                            rhs=pt[:, h, :],
                            start=(k == 0),
                            stop=(k == KT - 1),
                            skip_group_check=True,
                        )
                # epilogue: transpose + normalize + store
                for h in range(2):
                    ot = ot_pool.tile([65, QB], f16, tag="ot")
                    nc.vector.tensor_copy(ot[:], av[h][:])
                    for i in range(QB // 128):
                        tp = tr_ps.tile([128, 65], f16, tag="tp")
                        nc.tensor.transpose(tp[:], ot[:, ts(i, 128)], ident[:65, :65])
                        rc = small.tile([128, 1], f32, tag="rc")
                        nc.vector.reciprocal(rc[:], tp[:, 64:65])
                        res = small.tile([128, 64], f32, tag="res")
                        nc.vector.tensor_tensor(
                            res[:], tp[:, 0:64], rc[:, 0:1].to_broadcast([128, 64]), ALU.mult
                        )
                        nc.sync.dma_start(
                            out_d[ds(qb * QB + i * 128, 128), ds(h * 64, 64)],
                            res[:],
                        )

    nc.compile()
    return nc


def _host_prep(hidden_states, W_qkv, b_qkv, hebbian_trace):
    """Build per-core input maps (all fp16 except biases)."""
    h = np.asarray(hidden_states, np.float32).reshape(S, E)
    W = np.asarray(W_qkv, np.float32)
    b = np.asarray(b_qkv, np.float32)
    heb = np.asarray(hebbian_trace, np.float32)

    sc = HOST_SCALE  # scores become v = s_raw/(64*BETA*ln2); exp = exp(ACT_SCALE*v)
    hT = np.ascontiguousarray(h.T).astype(np.float16)  # [E, S]

    Wq, Wk, Wv = W[0:E], W[E : 2 * E], W[2 * E :]
    bq, bk, bv = b[0:E], b[E : 2 * E], b[2 * E :]
    hebd = np.ascontiguousarray(
        heb[:, np.arange(D), np.arange(D)]
    )  # [H, D] diagonal

    in_maps = []
    bv_scaled_full = (bv.reshape(H, D) * hebd).reshape(E)  # added on host at end
    for c in range(NCORES):
        hs = [2 * c, 2 * c + 1]
        rows = np.concatenate([np.arange(hh * D, (hh + 1) * D) for hh in hs])
        wq_c = (Wq[rows] * sc).astype(np.float32)           # [128, E]
        wk_c = (Wk[rows] * sc).astype(np.float32)
        wv_c = Wv[rows] * hebd[hs].reshape(128)[:, None]    # [128, E] hebbian folded
        # V layout per head: 64 dims + 1 zero col (becomes ones via memset)
        wv_ext = np.zeros((2 * 65, E), np.float32)
        wv_ext[0:64] = wv_c[0:64]
        wv_ext[65:129] = wv_c[64:128]
        in_maps.append(
            {
                "hT": hT,
                "wqT": np.ascontiguousarray(wq_c.T).astype(np.float16),
                "wkT": np.ascontiguousarray(wk_c.T).astype(np.float16),
                "wvT": np.ascontiguousarray(wv_ext.T).astype(np.float16),
                "bq": (bq[rows] * sc).astype(np.float32).reshape(128, 1),
                "bk": (bk[rows] * sc).astype(np.float32).reshape(128, 1),
            }
        )
    return in_maps, bv_scaled_full


def _get_nc():
    if "nc" not in _CACHE:
        _CACHE["nc"] = _build_program()
    return _CACHE["nc"]


def kernel(hidden_states, W_qkv, b_qkv, hebbian_trace, _trace=False, _tmpdir=None):
    from concourse.bass_utils import run_bass_kernel_spmd

    in_maps, bv_scaled = _host_prep(hidden_states, W_qkv, b_qkv, hebbian_trace)
    nc = _get_nc()
    res = run_bass_kernel_spmd(
        nc,
        in_maps,
        core_ids=list(range(NCORES)),
        trace=_trace,
        tmpdir=_tmpdir,
    )
    outs = [res.results[c]["out"] for c in range(NCORES)]
    full = np.concatenate(outs, axis=1).astype(np.float32)  # [S, E]
    full = full + bv_scaled[None, :]
    out = full.reshape(1, S, E)
    if _trace:
        return out, res
    return out
